# revision 11
# baseline (speedup 1.0000x reference)
"""Trainium2 Bass kernel for a KAN layer (piecewise-cubic spline edges).

y[b, j] = scale[j] * sum_i sum_p coeff[j, i, seg(x[b,i]), p] * t(x[b,i])^p

with 9 uniform segments on [-1, 1], t the within-segment coordinate.

Strategy:
  * Recast as one-hot-masked GEMM: y^T[j, b] = sum_{s,p,ichunk}
        coeffT[s,p,ichunk,:,j]^T @ (mask_s * t^p)[ichunk,:,b]
  * 8-way data parallel over batch (each core: 512 batch cols, full OUT).
  * Masked-power tiles built on DVE/ACT/GPSIMD, matmuls in float32r
    (fp32 stored, fp22 multiply, fp32 accumulate) at full PE rate.
"""

import numpy as np

import concourse.bass as bass
import concourse.mybir as mybir
from concourse import bacc
from concourse.tile import TileContext
from concourse.bass_utils import run_bass_kernel_spmd

AF = mybir.ActivationFunctionType
OP = mybir.AluOpType
F32 = mybir.dt.float32
F32R = mybir.dt.float32r

B, IN, OUT = 4096, 512, 512
S, P = 9, 4            # segments, polynomial terms
NC = 8                 # cores
NB = B // NC           # local batch (moving free dim)
ICH = IN // 128        # input chunks (contraction tiles)
JT = OUT // 128        # output-row tiles
UMAX = float(np.nextafter(np.float32(9.0), np.float32(0.0)))

# Tunables
AT_BUFS = 6            # in-flight masked-power tile groups
CT_BUFS = 4            # in-flight coeff tile groups

LAST_EXEC_NS = None
LAST_RESULTS = None
LAST_NC = None
LAST_IN_MAPS = None


def _build_nc():
    nc = bacc.Bacc("TRN2", target_bir_lowering=False, debug=False, num_devices=NC)

    xt_d = nc.dram_tensor("xt", [IN, NB], F32, kind="ExternalInput")
    cf_d = nc.dram_tensor("coeffr", [S * ICH, 128, P * JT * 128], F32R,
                          kind="ExternalInput")
    sc_d = nc.dram_tensor("scale", [OUT, 1], F32, kind="ExternalInput")
    yt_d = nc.dram_tensor("yt", [OUT, NB], F32, kind="ExternalOutput")

    with TileContext(nc) as tc:
        with (
            tc.tile_pool(name="xp", bufs=1) as xp,
            tc.tile_pool(name="atp", bufs=AT_BUFS) as atp,
            tc.tile_pool(name="ctp", bufs=CT_BUFS) as ctp,
            tc.tile_pool(name="outp", bufs=2) as outp,
            tc.tile_pool(name="pp", bufs=1, space="PSUM") as pp,
        ):
            xt_sb = xp.tile([128, ICH, NB], F32, name="xt_sb")
            nc.sync.dma_start(xt_sb, xt_d.rearrange("(c p) b -> p c b", p=128))
            sc_sb = xp.tile([128, JT, 1], F32, name="sc_sb")
            nc.sync.dma_start(sc_sb, sc_d.rearrange("(c p) o -> p c o", p=128))

            u_sb = xp.tile([128, ICH, NB], F32, name="u_sb")
            t_sb = xp.tile([128, ICH, NB], F32, name="t_sb")
            seg_sb = xp.tile([128, ICH, NB], F32, name="seg_sb")
            segi_sb = xp.tile([128, ICH, NB], mybir.dt.int32, name="segi_sb")

            for ic in range(ICH):
                xs = xt_sb[:, ic]
                us = u_sb[:, ic]
                ts = t_sb[:, ic]
                ss = seg_sb[:, ic]
                # u2 = clip(x,-1,1)*4.5 + 4.0 in [-0.5, 8.5]; RNE(u2) == floor
                # of the segment coordinate (verified exact vs searchsorted).
                nc.vector.tensor_scalar(us, xs, 1.0, -1.0, OP.min, OP.max)
                nc.vector.tensor_scalar(us, us, 4.5, 4.0, OP.mult, OP.add)
                nc.vector.tensor_copy(segi_sb[:, ic], us)            # RNE -> int32
                nc.vector.tensor_copy(ss, segi_sb[:, ic])            # back to f32
                # t = (u2 + 0.5) - seg
                nc.vector.scalar_tensor_tensor(ts, us, 0.5, ss, OP.add, OP.subtract)

            ps = [pp.tile([128, NB], F32, name=f"ps{jt}", tag=f"ps{jt}")
                  for jt in range(JT)]

            for s in range(S):
                for ic in range(ICH):
                    at = atp.tile([128, P, NB], F32R, name=f"at_{s}_{ic}", tag="at")
                    ts = t_sb[:, ic]
                    nc.vector.tensor_scalar(at[:, 0], seg_sb[:, ic], float(s), None,
                                            OP.is_equal)
                    nc.vector.tensor_mul(at[:, 1], at[:, 0], ts)
                    nc.scalar.activation(at[:, 2], at[:, 1], AF.Square)
                    nc.gpsimd.tensor_mul(at[:, 3], at[:, 2], at[:, 1])

                    ct = ctp.tile([128, P * JT * 128], F32R, name=f"ct_{s}_{ic}",
                                  tag="ct")
                    nc.sync.dma_start(ct, cf_d[s * ICH + ic])

                    at_r = at
                    ct_r = ct
                    first = (s == 0 and ic == 0)
                    last = (s == S - 1 and ic == ICH - 1)
                    for p in range(P):
                        for jt in range(JT):
                            nc.tensor.matmul(
                                ps[jt][:, :],
                                lhsT=ct_r[:, (p * JT + jt) * 128:(p * JT + jt + 1) * 128],
                                rhs=at_r[:, p, :],
                                start=(first and p == 0),
                                stop=(last and p == P - 1),
                            )

            for jt in range(JT):
                ot = outp.tile([128, NB], F32, name=f"ot{jt}", tag="ot")
                nc.scalar.activation(ot, ps[jt], AF.Copy, scale=sc_sb[:, jt])
                nc.sync.dma_start(yt_d[jt * 128:(jt + 1) * 128, :], ot)

    nc.compile()
    return nc


def kernel(x, coeff, scale, _trace=False):
    global LAST_EXEC_NS, LAST_RESULTS, LAST_NC, LAST_IN_MAPS
    x = np.ascontiguousarray(np.asarray(x, dtype=np.float32))
    coeff = np.ascontiguousarray(np.asarray(coeff, dtype=np.float32))
    scale = np.ascontiguousarray(np.asarray(scale, dtype=np.float32))

    # x^T shards: [IN, NB] per core
    xt = np.ascontiguousarray(x.T)
    # Round coeff to fp22 (e8m13, what the PE multiplies in) with RNE on the
    # host so the on-device f32r truncation is lossless.
    cb = coeff.view(np.uint32)
    cb = (cb + np.uint32(0x1FF) + ((cb >> np.uint32(10)) & np.uint32(1))) & \
        np.uint32(0xFFFFFC00)
    coeff = cb.view(np.float32)
    # coeff [OUT, IN, S, P] -> tiles [(s, ic), i_in, (p, jt, j_in)]
    cr = coeff.transpose(2, 3, 1, 0)                      # [S, P, IN, OUT]
    cr = cr.reshape(S, P, ICH, 128, JT, 128)              # s p ic i_in jt j_in
    cr = cr.transpose(0, 2, 3, 1, 4, 5)                   # s ic i_in p jt j_in
    cr = np.ascontiguousarray(cr.reshape(S * ICH, 128, P * JT * 128))
    sc2 = scale.reshape(OUT, 1)

    nc = _build_nc()
    in_maps = [
        {"xt": np.ascontiguousarray(xt[:, g * NB:(g + 1) * NB]),
         "coeffr": cr, "scale": sc2}
        for g in range(NC)
    ]
    res = run_bass_kernel_spmd(nc, in_maps, core_ids=list(range(NC)),
                               trace=_trace)
    LAST_RESULTS = res
    LAST_EXEC_NS = res.exec_time_ns
    LAST_NC = nc
    LAST_IN_MAPS = in_maps

    yt = np.concatenate([res.results[g]["yt"] for g in range(NC)], axis=1)
    return np.ascontiguousarray(yt.T)


# revision 20
# speedup vs baseline: 245.1478x; 245.1478x over previous
"""Trainium2 Bass kernel for a KAN layer (piecewise-cubic spline edges).

y[b, j] = scale[j] * sum_i sum_p coeff[j, i, seg(x[b,i]), p] * t(x[b,i])^p

with 9 uniform segments on [-1, 1], t the within-segment coordinate.

Strategy:
  * Recast as one-hot-masked GEMM: y^T[j, b] = sum_{s,p,ichunk}
        coeffT[s,p,ichunk,:,j]^T @ (mask_s * t^p)[ichunk,:,b]
  * 8-way data parallel over batch (each core: 512 batch cols, full OUT).
  * Masked-power tiles built on DVE/ACT/GPSIMD, matmuls in float32r
    (fp32 stored, fp22 multiply, fp32 accumulate) at full PE rate.
"""

import numpy as np

import concourse.bass as bass
import concourse.mybir as mybir
from concourse import bacc
from concourse.tile import TileContext
from concourse.bass_utils import run_bass_kernel_spmd

AF = mybir.ActivationFunctionType
OP = mybir.AluOpType
F32 = mybir.dt.float32
F32R = mybir.dt.float32r

B, IN, OUT = 4096, 512, 512
S, P = 9, 4            # segments, polynomial terms
NC = 8                 # cores
NB = B // NC           # local batch (moving free dim)
ICH = IN // 128        # input chunks (contraction tiles)
JT = OUT // 128        # output-row tiles
UMAX = float(np.nextafter(np.float32(9.0), np.float32(0.0)))

# Tunables
AT_BUFS = 6            # in-flight masked-power tile groups
CT_BUFS = 4            # in-flight coeff tile groups
MT3_ON_GPSIMD = False  # build t^3 tiles on GPSIMD (else VectorE)
DMA_BEFORE_AT = False  # emit coeff DMA before masked-power ops
JT_OUTER = True        # matmul inner loops: jt outer / p inner

LAST_EXEC_NS = None
LAST_RESULTS = None
LAST_NC = None
LAST_IN_MAPS = None


def _build_nc():
    nc = bacc.Bacc("TRN2", target_bir_lowering=False, debug=False, num_devices=NC)

    xt_d = nc.dram_tensor("xt", [IN, NB], F32, kind="ExternalInput")
    cf_d = nc.dram_tensor("coeffr", [S * ICH, 128, P * JT * 128], F32R,
                          kind="ExternalInput")
    sc_d = nc.dram_tensor("scale", [OUT, 1], F32, kind="ExternalInput")
    yt_d = nc.dram_tensor("yt", [OUT, NB], F32, kind="ExternalOutput")

    with TileContext(nc) as tc:
        with (
            tc.tile_pool(name="xp", bufs=1) as xp,
            tc.tile_pool(name="atp", bufs=AT_BUFS) as atp,
            tc.tile_pool(name="ctp", bufs=CT_BUFS) as ctp,
            tc.tile_pool(name="outp", bufs=2) as outp,
            tc.tile_pool(name="pp", bufs=1, space="PSUM") as pp,
        ):
            xt_sb = xp.tile([128, ICH, NB], F32, name="xt_sb")
            xt_r = xt_d.rearrange("(c p) b -> p c b", p=128)
            for ic in range(ICH):
                nc.sync.dma_start(xt_sb[:, ic], xt_r[:, ic])
            sc_sb = xp.tile([128, JT, 1], F32, name="sc_sb")
            nc.sync.dma_start(sc_sb, sc_d.rearrange("(c p) o -> p c o", p=128))

            u_sb = xp.tile([128, ICH, NB], F32, name="u_sb")
            t_sb = xp.tile([128, ICH, NB], F32, name="t_sb")
            seg_sb = xp.tile([128, ICH, NB], F32, name="seg_sb")
            segi_sb = xp.tile([128, ICH, NB], mybir.dt.int32, name="segi_sb")

            for ic in range(ICH):
                xs = xt_sb[:, ic]
                us = u_sb[:, ic]
                ts = t_sb[:, ic]
                ss = seg_sb[:, ic]
                # u2 = clip(x,-1,1)*4.5 + 4.0 in [-0.5, 8.5]; RNE(u2) == floor
                # of the segment coordinate (verified exact vs searchsorted).
                nc.vector.tensor_scalar(us, xs, 1.0, -1.0, OP.min, OP.max)
                nc.vector.tensor_scalar(us, us, 4.5, 4.0, OP.mult, OP.add)
                nc.vector.tensor_copy(segi_sb[:, ic], us)            # RNE -> int32
                nc.vector.tensor_copy(ss, segi_sb[:, ic])            # back to f32
                # t = (u2 + 0.5) - seg
                nc.vector.scalar_tensor_tensor(ts, us, 0.5, ss, OP.add, OP.subtract)

            ps = [pp.tile([128, NB], F32, name=f"ps{jt}", tag=f"ps{jt}")
                  for jt in range(JT)]

            for s in range(S):
                for ic in range(ICH):
                    at = atp.tile([128, P, NB], F32R, name=f"at_{s}_{ic}", tag="at")
                    ct = ctp.tile([128, JT, P * 128], F32R, name=f"ct_{s}_{ic}",
                                  tag="ct")
                    cf_g = cf_d[s * ICH + ic].rearrange("p (j q) -> p j q", j=JT)
                    if DMA_BEFORE_AT:
                        nc.sync.dma_start(ct, cf_g)
                    ts = t_sb[:, ic]
                    nc.vector.tensor_scalar(at[:, 0], seg_sb[:, ic], float(s), None,
                                            OP.is_equal)
                    nc.vector.tensor_mul(at[:, 1], at[:, 0], ts)
                    nc.scalar.activation(at[:, 2], at[:, 1], AF.Square)
                    eng3 = nc.gpsimd if MT3_ON_GPSIMD else nc.vector
                    eng3.tensor_mul(at[:, 3], at[:, 2], at[:, 1])
                    if not DMA_BEFORE_AT:
                        nc.sync.dma_start(ct, cf_g)

                    first = (s == 0 and ic == 0)
                    last = (s == S - 1 and ic == ICH - 1)
                    pjt = ([(p, jt) for jt in range(JT) for p in range(P)]
                           if JT_OUTER else
                           [(p, jt) for p in range(P) for jt in range(JT)])
                    for p, jt in pjt:
                        nc.tensor.matmul(
                            ps[jt][:, :],
                            lhsT=ct[:, jt, p * 128:(p + 1) * 128],
                            rhs=at[:, p, :],
                            start=(first and p == 0),
                            stop=(last and p == P - 1),
                        )

            for jt in range(JT):
                ot = outp.tile([128, NB], F32, name=f"ot{jt}", tag="ot")
                nc.scalar.activation(ot, ps[jt], AF.Copy, scale=sc_sb[:, jt])
                nc.sync.dma_start(yt_d[jt * 128:(jt + 1) * 128, :], ot)

    nc.compile()
    return nc


def kernel(x, coeff, scale, _trace=False):
    global LAST_EXEC_NS, LAST_RESULTS, LAST_NC, LAST_IN_MAPS
    x = np.ascontiguousarray(np.asarray(x, dtype=np.float32))
    coeff = np.ascontiguousarray(np.asarray(coeff, dtype=np.float32))
    scale = np.ascontiguousarray(np.asarray(scale, dtype=np.float32))

    # x^T shards: [IN, NB] per core
    xt = np.ascontiguousarray(x.T)
    # Round coeff to fp22 (e8m13, what the PE multiplies in) with RNE on the
    # host so the on-device f32r truncation is lossless.
    cb = coeff.view(np.uint32)
    cb = (cb + np.uint32(0x1FF) + ((cb >> np.uint32(10)) & np.uint32(1))) & \
        np.uint32(0xFFFFFC00)
    coeff = cb.view(np.float32)
    # coeff [OUT, IN, S, P] -> tiles [(s, ic), i_in, (p, jt, j_in)]
    cr = coeff.transpose(2, 3, 1, 0)                      # [S, P, IN, OUT]
    cr = cr.reshape(S, P, ICH, 128, JT, 128)              # s p ic i_in jt j_in
    cr = cr.transpose(0, 2, 3, 4, 1, 5)                   # s ic i_in jt p j_in
    cr = np.ascontiguousarray(cr.reshape(S * ICH, 128, P * JT * 128))
    sc2 = scale.reshape(OUT, 1)

    nc = _build_nc()
    in_maps = [
        {"xt": np.ascontiguousarray(xt[:, g * NB:(g + 1) * NB]),
         "coeffr": cr, "scale": sc2}
        for g in range(NC)
    ]
    res = run_bass_kernel_spmd(nc, in_maps, core_ids=list(range(NC)),
                               trace=_trace)
    LAST_RESULTS = res
    LAST_EXEC_NS = res.exec_time_ns
    LAST_NC = nc
    LAST_IN_MAPS = in_maps

    yt = np.concatenate([res.results[g]["yt"] for g in range(NC)], axis=1)
    return np.ascontiguousarray(yt.T)
